# revision 6
# baseline (speedup 1.0000x reference)
"""Trainium2 Bass kernel for nn_BatchRankingLoss (n=8192, 8 NeuronCores).

Math: reference computes sum over pairs i<j of relu(-(p_j-p_i)*sign(l_j-l_i) + 2).
The sum runs over UNORDERED pairs and is invariant to re-indexing, so we sort by
labels on the host: with q = preds[argsort(labels)], the loss becomes
    sum_{u<v} relu(2 + q_u - q_v)
(plus an exact O(#ties) host correction for tied labels, where sign()=0).

Device strategy (SPMD, 8 cores, one shared program):
- 64 row-tiles of 128 rows. Core k gets tiles {k+16m, 15-k+16m}, presented to the
  program as 8 fixed-width "slots" of [16,14,12,10,8,6,4,2] 512-col chunks.
  Each slot's column window starts at its diagonal block; unused tail columns are
  zero in the rhs data (both rhs rows 0 -> t=0 -> contributes 0 exactly).
- rhs data is packed into 8 "streams" on 16 SBUF partitions ([16, 4608] bf16) so
  the load DMA covers 16 partitions; K=16 matmuls with zero-padded lhsT lanes
  select the stream (pair of partitions) each chunk lives on.
- PE: one K=16 bf16 matmul per 512-col chunk produces t = q_u + (2 - q_v) in
  f32 PSUM; the diagonal chunk gets a second accumulating matmul adding -1e9 on
  the lower triangle (tri/pen constant operands), masking j<=i pairs.
- ACT lane: activation(Relu, accum_out) -> per-partition sum of relu(t).
- DVE lane: tensor_reduce(add, |.|) -> per-partition sum of |t|; combined with
  the analytic linear term sum(t) (per-slot affine in q_u, per-core inputs) via
  relu(t) = (t + |t|)/2.
- Each core outputs a [128,1] partial; host sums 8x128 partials + tie correction.
"""

import numpy as np

N = 8192
NBLK = 64                                     # 128-row tiles
SLOT_CHUNKS = [16, 14, 12, 10, 8, 6, 4, 2]    # 512-col chunks per slot
NCHUNKS = sum(SLOT_CHUNKS)                    # 72
STREAM_CAP = 9                                # chunks per stream (9*512 = 4608)
GROUP_CHUNKS = 2                              # chunks per reduce group (2 banks)
PENALTY = -1.0e9

# ---------------------------------------------------------------------------
# Stream packing: (slot, chunk) -> (stream, pos); lhsT variant per (slot,stream)
# ---------------------------------------------------------------------------

def _pack_streams():
    chunk_map = {}           # (slot, chunk_idx) -> (stream, pos)
    variants = []            # list of (slot, stream); index = lhsT variant id
    vmap = {}
    stream = 0
    pos = 0
    for s, nch in enumerate(SLOT_CHUNKS):
        for c in range(nch):
            if pos == STREAM_CAP:
                stream += 1
                pos = 0
            chunk_map[(s, c)] = (stream, pos)
            if (s, stream) not in vmap:
                vmap[(s, stream)] = len(variants)
                variants.append((s, stream))
            pos += 1
    assert stream == 7 and pos == STREAM_CAP
    return chunk_map, variants, vmap

CHUNK_MAP, VARIANTS, VMAP = _pack_streams()
NVAR = len(VARIANTS)

# ---------------------------------------------------------------------------
# Group schedule
# ---------------------------------------------------------------------------

def make_schedule():
    """Groups: (slot, chunk0, nchunks, is_diag, engine 'A'|'D')."""
    groups = []
    for s, nch in enumerate(SLOT_CHUNKS):
        c = 0
        while c < nch:
            g = min(GROUP_CHUNKS, nch - c)
            groups.append([s, c, g, c == 0])
            c += g
    act_cost = 0.0
    dve_cost = 0.0
    sched = []
    for s, c0, g, diag in groups:
        w = g * 512
        ca = w * 0.8333 + 290.0
        cd = w * 1.0417 + 170.0
        if diag:
            eng = "A"
        else:
            eng = "A" if act_cost + ca <= dve_cost + cd else "D"
        if eng == "A":
            act_cost += ca
        else:
            dve_cost += cd
        sched.append((s, c0, g, diag, eng))
    return sched

SCHEDULE = make_schedule()

# ---------------------------------------------------------------------------
# Device program
# ---------------------------------------------------------------------------

_CACHE = {}

def build_program():
    import concourse.bacc as bacc
    import concourse.mybir as mybir
    from concourse.tile import TileContext

    F32 = mybir.dt.float32
    BF16 = mybir.dt.bfloat16
    AX = mybir.AxisListType
    OP = mybir.AluOpType
    AF = mybir.ActivationFunctionType

    nA = sum(1 for g in SCHEDULE if g[4] == "A")
    nD = sum(1 for g in SCHEDULE if g[4] == "D")

    nc = bacc.Bacc(trn_type="TRN2")
    rhs_d = nc.dram_tensor("rhs", [16, STREAM_CAP * 512], BF16, kind="ExternalInput")
    lhs_d = nc.dram_tensor("lhs", [16, NVAR * 128], BF16, kind="ExternalInput")
    tri_d = nc.dram_tensor("tri", [128, 128], BF16, kind="ExternalInput")
    pen_d = nc.dram_tensor("pen", [128, 512], BF16, kind="ExternalInput")
    qcol_d = nc.dram_tensor("qcol", [128, 8], F32, kind="ExternalInput")
    lin_d = nc.dram_tensor("linab", [128, 16], F32, kind="ExternalInput")
    out_d = nc.dram_tensor("out", [128, 1], F32, kind="ExternalOutput")

    with TileContext(nc) as tc:
        with tc.tile_pool(name="consts", bufs=1) as cpool, \
             tc.tile_pool(name="scr", bufs=2) as spool, \
             tc.tile_pool(name="ps", bufs=4, space="PSUM") as psp:
            RHS = cpool.tile([16, STREAM_CAP * 512], BF16)
            LHS = cpool.tile([16, NVAR * 128], BF16)
            TRI = cpool.tile([128, 128], BF16)
            PEN = cpool.tile([128, 512], BF16)
            QCOL = cpool.tile([128, 8], F32)
            LIN = cpool.tile([128, 16], F32)
            ACCA = cpool.tile([128, nA], F32)
            ACCD = cpool.tile([128, nD], F32)
            ACCL = cpool.tile([128, 8], F32)
            R = cpool.tile([128, 4], F32)
            OUT = cpool.tile([128, 1], F32)

            # diag chunks live at low stream positions: load those cols first
            # so compute can start while the rest streams in.
            nc.sync.dma_start(out=RHS[:, :1024], in_=rhs_d[:, :1024])
            nc.sync.dma_start(out=RHS[:, 1024:], in_=rhs_d[:, 1024:])
            nc.sync.dma_start(out=LHS[:], in_=lhs_d[:])
            nc.sync.dma_start(out=TRI[:], in_=tri_d[:])
            nc.sync.dma_start(out=PEN[:], in_=pen_d[:])
            nc.sync.dma_start(out=QCOL[:], in_=qcol_d[:])
            nc.sync.dma_start(out=LIN[:], in_=lin_d[:])

            # PE p-state warmup: ~4us of dummy matmuls on an uninitialized
            # tile (no data deps) so the tensor engine ramps to max clock
            # while the input DMAs are still in flight.
            DW = cpool.tile([128, 512], BF16)
            nc.gpsimd.memset(DW[:], 0.0)
            WPS = psp.tile([128, GROUP_CHUNKS * 512], F32, tag="ps")
            for i in range(10):
                nc.tensor.matmul(WPS[:, 0:512], DW[0:16, 0:128], DW[0:16, 0:512],
                                 start=True, stop=True)

            ia = 0
            id_ = 0
            for (s, c0, g, diag, eng) in SCHEDULE:
                w = g * 512
                PS = psp.tile([128, GROUP_CHUNKS * 512], F32, tag="ps")
                for b in range(g):
                    st, pos = CHUNK_MAP[(s, c0 + b)]
                    v = VMAP[(s, st)]
                    nc.tensor.matmul(PS[:, b * 512:(b + 1) * 512],
                                     LHS[:, v * 128:(v + 1) * 128],
                                     RHS[:, pos * 512:(pos + 1) * 512],
                                     start=True, stop=not (diag and b == 0))
                if diag:
                    nc.tensor.matmul(PS[:, 0:512], TRI[:], PEN[:],
                                     start=False, stop=True)
                if eng == "A":
                    SCR = spool.tile([128, GROUP_CHUNKS * 512], F32, tag="scr")
                    nc.scalar.activation(out=SCR[:, :w], in_=PS[:, :w], func=AF.Relu,
                                         bias=0.0, scale=1.0,
                                         accum_out=ACCA[:, ia:ia + 1])
                    ia += 1
                else:
                    nc.vector.tensor_reduce(out=ACCD[:, id_:id_ + 1], in_=PS[:, :w],
                                            axis=AX.X, op=OP.add,
                                            apply_absolute_value=True)
                    id_ += 1

            # linear terms: accL[:, s] = A_s * q_u + B_s   (A,B per-core inputs)
            for s in range(8):
                nc.vector.tensor_scalar(ACCL[:, s:s + 1], QCOL[:, s:s + 1],
                                        LIN[:, 2 * s:2 * s + 1],
                                        LIN[:, 2 * s + 1:2 * s + 2],
                                        OP.mult, OP.add)

            # combine: out = sum(ACCA) + 0.5*(sum(ACCD) + sum(ACCL))
            nc.vector.tensor_reduce(out=R[:, 0:1], in_=ACCA[:], axis=AX.X, op=OP.add)
            nc.vector.tensor_reduce(out=R[:, 1:2], in_=ACCD[:], axis=AX.X, op=OP.add)
            nc.vector.tensor_reduce(out=R[:, 2:3], in_=ACCL[:], axis=AX.X, op=OP.add)
            nc.vector.tensor_tensor(out=R[:, 1:2], in0=R[:, 1:2], in1=R[:, 2:3],
                                    op=OP.add)
            nc.vector.tensor_scalar(R[:, 1:2], R[:, 1:2], 0.5, None, OP.mult)
            nc.vector.tensor_tensor(out=R[:, 0:1], in0=R[:, 0:1], in1=R[:, 1:2],
                                    op=OP.add)
            nc.vector.tensor_copy(out=OUT[:], in_=R[:, 0:1])
            nc.sync.dma_start(out=out_d[:], in_=OUT[:])

    nc.finalize()
    return nc


def get_program():
    if "nc" not in _CACHE:
        _CACHE["nc"] = build_program()
    return _CACHE["nc"]

# ---------------------------------------------------------------------------
# Host side
# ---------------------------------------------------------------------------

def core_tiles(k):
    """Row-tiles for core k, widest first (matches slot widths)."""
    return sorted([k + 16 * m for m in range(4)] + [15 - k + 16 * m for m in range(4)])


def build_inputs(q):
    """Per-core in_maps for label-sorted preds q (np.float32 [8192])."""
    import ml_dtypes
    BF = ml_dtypes.bfloat16
    q = q.astype(np.float32)
    qb = q.astype(BF)                                     # device-visible q
    rhs1_full = (2.0 - q).astype(np.float32).astype(BF)   # bf16(2 - q_v)
    tri = np.triu(np.ones((128, 128), np.float32)).astype(BF)
    pen = np.zeros((128, 512), np.float32)
    pen[np.arange(128), np.arange(128)] = PENALTY
    pen = pen.astype(BF)

    in_maps = []
    for k in range(8):
        tiles = core_tiles(k)
        rhs = np.zeros((16, STREAM_CAP * 512), BF)
        lhs = np.zeros((16, NVAR * 128), BF)
        qcol = np.zeros((128, 8), np.float32)
        lin = np.zeros((128, 16), np.float32)
        for s, t in enumerate(tiles):
            real = (NBLK - t) * 128                  # real window width in cols
            qcol[:, s] = qb[t * 128:(t + 1) * 128].astype(np.float32)
            # scatter this slot's window into its stream chunks
            for c in range(SLOT_CHUNKS[s]):
                st, pos = CHUNK_MAP[(s, c)]
                lo = c * 512
                take = min(max(real - lo, 0), 512)
                if take > 0:
                    rhs[2 * st, pos * 512: pos * 512 + take] = np.float32(1.0)
                    rhs[2 * st + 1, pos * 512: pos * 512 + take] = \
                        rhs1_full[t * 128 + lo: t * 128 + lo + take]
                # lhsT variant for this (slot, stream)
                v = VMAP[(s, st)]
                lhs[2 * st, v * 128:(v + 1) * 128] = qb[t * 128:(t + 1) * 128]
                lhs[2 * st + 1, v * 128:(v + 1) * 128] = np.float32(1.0)
            # linear terms over this slot's DVE groups (f64 accumulate)
            A = 0.0
            B = 0.0
            for (gs, c0, g, diag, eng) in SCHEDULE:
                if gs != s or eng != "D":
                    continue
                for b in range(g):
                    st, pos = CHUNK_MAP[(s, c0 + b)]
                    A += rhs[2 * st, pos * 512:(pos + 1) * 512].astype(np.float64).sum()
                    B += rhs[2 * st + 1, pos * 512:(pos + 1) * 512].astype(np.float64).sum()
            lin[:, 2 * s] = np.float32(A)
            lin[:, 2 * s + 1] = np.float32(B)
        in_maps.append({"rhs": rhs, "lhs": lhs, "tri": tri, "pen": pen,
                        "qcol": qcol, "linab": lin})
    return in_maps


def tie_correction(labels, q, order):
    """Exact correction for tied labels: reference uses sign()=0 there."""
    ls = labels[order]
    corr = 0.0
    i = 0
    n = len(ls)
    while i < n:
        j = i + 1
        while j < n and ls[j] == ls[i]:
            j += 1
        if j - i > 1:
            for u in range(i, j):
                for v in range(u + 1, j):
                    corr += 2.0 - max(0.0, 2.0 + float(q[u]) - float(q[v]))
        i = j
    return corr


def run(inputs, trace=False):
    from concourse.bass_utils import run_bass_kernel_spmd

    preds = np.asarray(inputs["preds"], dtype=np.float32)
    labels = np.asarray(inputs["labels"], dtype=np.float32)
    order = np.argsort(labels, kind="stable")
    q = preds[order]

    nc = get_program()
    in_maps = build_inputs(q)
    res = run_bass_kernel_spmd(nc, in_maps, core_ids=list(range(8)), trace=trace)
    total = 0.0
    for c in range(8):
        total += res.results[c]["out"].astype(np.float64).sum()
    total += tie_correction(labels, q, order)
    return np.float32(total), res


def kernel(**inputs):
    out, _ = run(inputs, trace=False)
    return out


# revision 7
# speedup vs baseline: 1.1179x; 1.1179x over previous
"""Trainium2 Bass kernel for nn_BatchRankingLoss (n=8192, 8 NeuronCores).

Math: reference computes sum over pairs i<j of relu(-(p_j-p_i)*sign(l_j-l_i) + 2).
The sum runs over UNORDERED pairs and is invariant to re-indexing, so we sort by
labels on the host: with q = preds[argsort(labels)], the loss becomes
    sum_{u<v} relu(2 + q_u - q_v)
(plus an exact O(#ties) host correction for tied labels, where sign()=0).

Device strategy (SPMD, 8 cores, one shared program). 64 row-tiles of 128 rows;
core k owns tiles {k+16m, 15-k+16m}, presented as 8 fixed-width "slots" of
[16,14,12,10,8,6,4,2] 512-col chunks (window starts at the diagonal block;
unused tail columns zero-padded). Three engines are saturated in parallel:

- PE route (40 chunks/slot-proportional, incl. every diagonal chunk):
  K=16 bf16 matmul per 512-col chunk -> t = q_u + (2 - q_v) in f32 PSUM (rhs
  packed into 8 partition-pair "streams", zero lhsT lanes select the stream).
  Diagonal chunks get a second [128,128] matmul adding -1e9 on the lower
  triangle. PSUM groups are reduced by either:
    ACT: activation(Relu, accum_out) -> sum relu(t)
    DVE: tensor_reduce(add, abs) -> sum |t|, combined with the analytic linear
         term sum(t) (affine in q_u, per-core inputs) via relu = (t + |t|)/2.
- ACT-direct route (32 chunks): a broadcast tile QB[128, 16384] holds
  bf16(2 - q_v) replicated across partitions; activation(Relu, bias=q_u,
  accum_out) computes sum_v relu(2 - q_v + q_u) in ONE ACT pass (no PE, no
  separate reduce). Padded columns hold -1000 so relu kills them.

Each core outputs a [128,1] partial; host sums 8x128 partials + tie correction.
"""

import numpy as np

N = 8192
NBLK = 64
SLOT_CHUNKS = [16, 14, 12, 10, 8, 6, 4, 2]    # 512-col chunks per slot
PE_CHUNKS = [9, 8, 7, 6, 4, 3, 2, 1]          # chunks on the PE route per slot
ALT_CHUNKS = [c - p for c, p in zip(SLOT_CHUNKS, PE_CHUNKS)]   # ACT-direct
N_PE = sum(PE_CHUNKS)                          # 40
N_ALT = sum(ALT_CHUNKS)                        # 32
STREAM_CAP = 5                                 # PE chunks per stream (5*512)
QB_COLS = N_ALT * 512                          # 16384
PENALTY = -1.0e9
PAD_VAL = -1000.0

# ---------------------------------------------------------------------------
# Stream packing for the PE route
# ---------------------------------------------------------------------------

def _pack_streams():
    chunk_map = {}
    variants = []
    vmap = {}
    stream = 0
    pos = 0
    for s, nch in enumerate(PE_CHUNKS):
        for c in range(nch):
            if pos == STREAM_CAP:
                stream += 1
                pos = 0
            chunk_map[(s, c)] = (stream, pos)
            if (s, stream) not in vmap:
                vmap[(s, stream)] = len(variants)
                variants.append((s, stream))
            pos += 1
    assert stream == 7 and pos == STREAM_CAP, (stream, pos)
    return chunk_map, variants, vmap

CHUNK_MAP, VARIANTS, VMAP = _pack_streams()
NVAR = len(VARIANTS)

# ALT segment offsets in QB (per slot), in columns
ALT_OFFS = []
_o = 0
for _c in ALT_CHUNKS:
    ALT_OFFS.append(_o)
    _o += _c * 512
assert _o == QB_COLS

# ---------------------------------------------------------------------------
# Schedule: PE-route reduce groups + engine assignment
# ---------------------------------------------------------------------------

def make_schedule():
    """PE-route groups: (slot, chunk0, nchunks, is_diag, engine)."""
    groups = []
    for s, nch in enumerate(PE_CHUNKS):
        c = 0
        while c < nch:
            g = min(2, nch - c)
            groups.append([s, c, g, c == 0])
            c += g
    # diag groups forced to ACT; others balance DVE-heavy (ACT also runs the
    # ACT-direct route, so give DVE everything it can take)
    act_cost = sum(ALT_CHUNKS) * 512 * 0.8333 + 8 * 370.0   # ACT-direct load
    dve_cost = 0.0
    sched = []
    for s, c0, g, diag in groups:
        w = g * 512
        ca = w * 0.8333 + 290.0
        cd = w * 1.0417 + 170.0
        if diag:
            eng = "A"
        else:
            eng = "A" if act_cost + ca <= dve_cost + cd else "D"
        if eng == "A":
            act_cost += ca
        else:
            dve_cost += cd
        sched.append((s, c0, g, diag, eng))
    return sched

SCHEDULE = make_schedule()

# ---------------------------------------------------------------------------
# Device program
# ---------------------------------------------------------------------------

_CACHE = {}

def build_program():
    import concourse.bacc as bacc
    import concourse.mybir as mybir
    from concourse.tile import TileContext

    F32 = mybir.dt.float32
    BF16 = mybir.dt.bfloat16
    AX = mybir.AxisListType
    OP = mybir.AluOpType
    AF = mybir.ActivationFunctionType

    nA = sum(1 for g in SCHEDULE if g[4] == "A") + 8   # + 8 ACT-direct groups
    nD = sum(1 for g in SCHEDULE if g[4] == "D")

    nc = bacc.Bacc(trn_type="TRN2")
    rhs_d = nc.dram_tensor("rhs", [16, STREAM_CAP * 512], BF16, kind="ExternalInput")
    lhs_d = nc.dram_tensor("lhs", [16, NVAR * 128], BF16, kind="ExternalInput")
    tri_d = nc.dram_tensor("tri", [128, 128], BF16, kind="ExternalInput")
    pen_d = nc.dram_tensor("pen", [128, 128], BF16, kind="ExternalInput")
    qb_d = nc.dram_tensor("qb", [128, QB_COLS], BF16, kind="ExternalInput")
    qcol_d = nc.dram_tensor("qcol", [128, 8], F32, kind="ExternalInput")
    lin_d = nc.dram_tensor("linab", [128, 16], F32, kind="ExternalInput")
    out_d = nc.dram_tensor("out", [128, 1], F32, kind="ExternalOutput")

    with TileContext(nc) as tc:
        with tc.tile_pool(name="consts", bufs=1) as cpool, \
             tc.tile_pool(name="scr", bufs=2) as spool, \
             tc.tile_pool(name="ps", bufs=4, space="PSUM") as psp:
            RHS = cpool.tile([16, STREAM_CAP * 512], BF16)
            LHS = cpool.tile([16, NVAR * 128], BF16)
            TRI = cpool.tile([128, 128], BF16)
            PEN = cpool.tile([128, 128], BF16)
            QB = cpool.tile([128, QB_COLS], BF16)
            QCOL = cpool.tile([128, 8], F32)
            LIN = cpool.tile([128, 16], F32)
            ACCA = cpool.tile([128, nA], F32)
            ACCD = cpool.tile([128, max(nD, 1)], F32)
            ACCL = cpool.tile([128, 8], F32)
            R = cpool.tile([128, 4], F32)
            OUT = cpool.tile([128, 1], F32)

            nc.sync.dma_start(out=RHS[:], in_=rhs_d[:])
            nc.sync.dma_start(out=LHS[:], in_=lhs_d[:])
            nc.sync.dma_start(out=TRI[:], in_=tri_d[:])
            nc.sync.dma_start(out=PEN[:], in_=pen_d[:])
            nc.sync.dma_start(out=QCOL[:], in_=qcol_d[:])
            nc.sync.dma_start(out=LIN[:], in_=lin_d[:])
            # QB streamed per-slot so ACT-direct groups start early
            for s in range(8):
                w = ALT_CHUNKS[s] * 512
                if w:
                    nc.sync.dma_start(out=QB[:, ALT_OFFS[s]:ALT_OFFS[s] + w],
                                      in_=qb_d[:, ALT_OFFS[s]:ALT_OFFS[s] + w])

            ia = 0
            id_ = 0
            alt_done = [False] * 8
            for gi, (s, c0, g, diag, eng) in enumerate(SCHEDULE):
                w = g * 512
                PS = psp.tile([128, 1024], F32, tag="ps")
                for b in range(g):
                    st, pos = CHUNK_MAP[(s, c0 + b)]
                    v = VMAP[(s, st)]
                    nc.tensor.matmul(PS[:, b * 512:(b + 1) * 512],
                                     LHS[:, v * 128:(v + 1) * 128],
                                     RHS[:, pos * 512:(pos + 1) * 512],
                                     start=True, stop=not (diag and b == 0))
                if diag:
                    nc.tensor.matmul(PS[:, 0:128], TRI[:], PEN[:],
                                     start=False, stop=True)
                if eng == "A":
                    SCR = spool.tile([128, 1024], F32, tag="scr")
                    nc.scalar.activation(out=SCR[:, :w], in_=PS[:, :w], func=AF.Relu,
                                         bias=0.0, scale=1.0,
                                         accum_out=ACCA[:, ia:ia + 1])
                    ia += 1
                else:
                    nc.vector.tensor_reduce(out=ACCD[:, id_:id_ + 1], in_=PS[:, :w],
                                            axis=AX.X, op=OP.add,
                                            apply_absolute_value=True)
                    id_ += 1
                # interleave ACT-direct groups after this slot's PE groups
                if not alt_done[s]:
                    last_of_slot = all(SCHEDULE[j][0] != s for j in
                                       range(gi + 1, len(SCHEDULE)))
                    if last_of_slot and ALT_CHUNKS[s] > 0:
                        wq = ALT_CHUNKS[s] * 512
                        SCR2 = spool.tile([128, 4096], F32, tag="scr2")
                        nc.scalar.activation(out=SCR2[:, :wq],
                                             in_=QB[:, ALT_OFFS[s]:ALT_OFFS[s] + wq],
                                             func=AF.Relu,
                                             bias=QCOL[:, s:s + 1], scale=1.0,
                                             accum_out=ACCA[:, ia:ia + 1])
                        ia += 1
                        alt_done[s] = True

            # linear terms: accL[:, s] = A_s * q_u + B_s
            for s in range(8):
                nc.vector.tensor_scalar(ACCL[:, s:s + 1], QCOL[:, s:s + 1],
                                        LIN[:, 2 * s:2 * s + 1],
                                        LIN[:, 2 * s + 1:2 * s + 2],
                                        OP.mult, OP.add)

            # combine: out = sum(ACCA) + 0.5*(sum(ACCD) + sum(ACCL))
            nc.vector.tensor_reduce(out=R[:, 0:1], in_=ACCA[:], axis=AX.X, op=OP.add)
            nc.vector.tensor_reduce(out=R[:, 1:2], in_=ACCD[:], axis=AX.X, op=OP.add)
            nc.vector.tensor_reduce(out=R[:, 2:3], in_=ACCL[:], axis=AX.X, op=OP.add)
            nc.vector.tensor_tensor(out=R[:, 1:2], in0=R[:, 1:2], in1=R[:, 2:3],
                                    op=OP.add)
            nc.vector.tensor_scalar(R[:, 1:2], R[:, 1:2], 0.5, None, OP.mult)
            nc.vector.tensor_tensor(out=R[:, 0:1], in0=R[:, 0:1], in1=R[:, 1:2],
                                    op=OP.add)
            nc.vector.tensor_copy(out=OUT[:], in_=R[:, 0:1])
            nc.sync.dma_start(out=out_d[:], in_=OUT[:])

    nc.finalize()
    return nc


def get_program():
    if "nc" not in _CACHE:
        _CACHE["nc"] = build_program()
    return _CACHE["nc"]

# ---------------------------------------------------------------------------
# Host side
# ---------------------------------------------------------------------------

def core_tiles(k):
    return sorted([k + 16 * m for m in range(4)] + [15 - k + 16 * m for m in range(4)])


def build_inputs(q):
    """Per-core in_maps for label-sorted preds q (np.float32 [8192])."""
    import ml_dtypes
    BF = ml_dtypes.bfloat16
    q = q.astype(np.float32)
    qb16 = q.astype(BF)
    rhs1_full = (2.0 - q).astype(np.float32).astype(BF)
    tri = np.triu(np.ones((128, 128), np.float32)).astype(BF)
    pen = np.zeros((128, 128), np.float32)
    pen[np.arange(128), np.arange(128)] = PENALTY
    pen = pen.astype(BF)

    in_maps = []
    for k in range(8):
        tiles = core_tiles(k)
        rhs = np.zeros((16, STREAM_CAP * 512), BF)
        lhs = np.zeros((16, NVAR * 128), BF)
        qbt = np.full((128, QB_COLS), PAD_VAL, np.float32).astype(BF)
        qcol = np.zeros((128, 8), np.float32)
        lin = np.zeros((128, 16), np.float32)
        for s, t in enumerate(tiles):
            real = (NBLK - t) * 128
            qcol[:, s] = qb16[t * 128:(t + 1) * 128].astype(np.float32)
            # PE-route chunks
            for c in range(PE_CHUNKS[s]):
                st, pos = CHUNK_MAP[(s, c)]
                lo = c * 512
                take = min(max(real - lo, 0), 512)
                if take > 0:
                    rhs[2 * st, pos * 512: pos * 512 + take] = np.float32(1.0)
                    rhs[2 * st + 1, pos * 512: pos * 512 + take] = \
                        rhs1_full[t * 128 + lo: t * 128 + lo + take]
                v = VMAP[(s, st)]
                lhs[2 * st, v * 128:(v + 1) * 128] = qb16[t * 128:(t + 1) * 128]
                lhs[2 * st + 1, v * 128:(v + 1) * 128] = np.float32(1.0)
            # ACT-direct chunks (tail of the window)
            for a in range(ALT_CHUNKS[s]):
                lo = (PE_CHUNKS[s] + a) * 512
                take = min(max(real - lo, 0), 512)
                col0 = ALT_OFFS[s] + a * 512
                if take > 0:
                    qbt[:, col0:col0 + take] = \
                        rhs1_full[t * 128 + lo: t * 128 + lo + take][None, :]
            # linear terms over this slot's DVE groups
            A = 0.0
            B = 0.0
            for (gs, c0, g, diag, eng) in SCHEDULE:
                if gs != s or eng != "D":
                    continue
                for b in range(g):
                    st, pos = CHUNK_MAP[(s, c0 + b)]
                    A += rhs[2 * st, pos * 512:(pos + 1) * 512].astype(np.float64).sum()
                    B += rhs[2 * st + 1, pos * 512:(pos + 1) * 512].astype(np.float64).sum()
            lin[:, 2 * s] = np.float32(A)
            lin[:, 2 * s + 1] = np.float32(B)
        in_maps.append({"rhs": rhs, "lhs": lhs, "tri": tri, "pen": pen,
                        "qb": qbt, "qcol": qcol, "linab": lin})
    return in_maps


def emulate(in_maps):
    """Numpy emulation of the device program (for offline validation)."""
    total = 0.0
    for k in range(8):
        m = in_maps[k]
        rhs = m["rhs"].astype(np.float32)
        lhs = m["lhs"].astype(np.float32)
        tri = m["tri"].astype(np.float32)
        pen = m["pen"].astype(np.float32)
        qb = m["qb"].astype(np.float32)
        qcol = m["qcol"]
        lin = m["linab"]
        accA = 0.0
        accD = 0.0
        accL = 0.0
        for (s, c0, g, diag, eng) in SCHEDULE:
            ps = np.zeros((128, g * 512), np.float64)
            for b in range(g):
                st, pos = CHUNK_MAP[(s, c0 + b)]
                v = VMAP[(s, st)]
                L = lhs[:, v * 128:(v + 1) * 128]
                Rr = rhs[:, pos * 512:(pos + 1) * 512]
                ps[:, b * 512:(b + 1) * 512] = L.T @ Rr
            if diag:
                ps[:, 0:128] += tri.T @ pen
            if eng == "A":
                accA += np.maximum(ps, 0).sum()
            else:
                accD += np.abs(ps).sum()
        for s in range(8):
            wq = ALT_CHUNKS[s] * 512
            if wq:
                t = qb[:, ALT_OFFS[s]:ALT_OFFS[s] + wq] + qcol[:, s][:, None]
                accA += np.maximum(t, 0).sum()
            accL += (lin[0, 2 * s] * qcol[:, s] + lin[0, 2 * s + 1]).sum()
        total += accA + 0.5 * (accD + accL)
    return total


def tie_correction(labels, q, order):
    ls = labels[order]
    corr = 0.0
    i = 0
    n = len(ls)
    while i < n:
        j = i + 1
        while j < n and ls[j] == ls[i]:
            j += 1
        if j - i > 1:
            for u in range(i, j):
                for v in range(u + 1, j):
                    corr += 2.0 - max(0.0, 2.0 + float(q[u]) - float(q[v]))
        i = j
    return corr


def run(inputs, trace=False):
    from concourse.bass_utils import run_bass_kernel_spmd

    preds = np.asarray(inputs["preds"], dtype=np.float32)
    labels = np.asarray(inputs["labels"], dtype=np.float32)
    order = np.argsort(labels, kind="stable")
    q = preds[order]

    nc = get_program()
    in_maps = build_inputs(q)
    res = run_bass_kernel_spmd(nc, in_maps, core_ids=list(range(8)), trace=trace)
    total = 0.0
    for c in range(8):
        total += res.results[c]["out"].astype(np.float64).sum()
    total += tie_correction(labels, q, order)
    return np.float32(total), res


def kernel(**inputs):
    out, _ = run(inputs, trace=False)
    return out


# revision 14
# speedup vs baseline: 1.2401x; 1.1093x over previous
"""Trainium2 Bass kernel for nn_BatchRankingLoss (n=8192, 8 NeuronCores).

Math: reference computes sum over pairs i<j of relu(-(p_j-p_i)*sign(l_j-l_i) + 2).
The sum runs over UNORDERED pairs and is invariant to re-indexing, so we sort by
labels on the host: with q = preds[argsort(labels)], the loss becomes
    sum_{u<v} relu(2 + q_u - q_v)
(plus an exact O(#ties) host correction for tied labels, where sign()=0).

Device strategy (SPMD, 8 cores, one shared program). 64 row-tiles of 128 rows;
core k owns tiles {k+16m, 15-k+16m}, presented as 8 fixed-width "slots" of
[16,14,12,10,8,6,4,2] 512-col chunks (window starts at the diagonal block;
unused tail columns zero-padded). Three engines are saturated in parallel:

- PE route (46 chunks, slot-proportional, incl. every diagonal chunk):
  K=16 bf16 matmul per 512-col chunk -> t = q_u + (2 - q_v) in f32 PSUM (rhs
  packed into 8 partition-pair "streams", zero lhsT lanes select the stream).
  Diagonal chunks get a second [128,128] matmul adding -1e9 on the lower
  triangle. PSUM groups are reduced by either:
    ACT: activation(Relu, accum_out) -> sum relu(t)
    DVE: tensor_reduce(add, abs) -> sum |t|, combined with the analytic linear
         term sum(t) (affine in q_u, per-core inputs) via relu = (t + |t|)/2.
- ACT-direct route (26 chunks): a broadcast tile QB[128, 13312] holds
  bf16(2 - q_v) replicated across partitions; activation(Relu, bias=q_u,
  accum_out) computes sum_v relu(2 - q_v + q_u) in ONE ACT pass (no PE, no
  separate reduce). Padded columns hold -1000 so relu kills them.

Each core outputs a [128,1] partial; host sums 8x128 partials + tie correction.
"""

import numpy as np

N = 8192
NBLK = 64
SLOT_CHUNKS = [16, 14, 12, 10, 8, 6, 4, 2]    # 512-col chunks per slot
PE_CHUNKS = [9, 8, 7, 6, 4, 3, 2, 1]          # chunks on the PE route per slot
ALT_CHUNKS = [c - p for c, p in zip(SLOT_CHUNKS, PE_CHUNKS)]   # ACT-direct
N_PE = sum(PE_CHUNKS)                          # 40
N_ALT = sum(ALT_CHUNKS)                        # 32
STREAM_CAP = 6                                 # PE chunks per stream (6*512)
QB_COLS = N_ALT * 512                          # 16384
PENALTY = -1.0e9
PAD_VAL = -1000.0

# ---------------------------------------------------------------------------
# Stream packing for the PE route
# ---------------------------------------------------------------------------

def _pack_streams():
    chunk_map = {}
    variants = []
    vmap = {}
    stream = 0
    pos = 0
    for s, nch in enumerate(PE_CHUNKS):
        for c in range(nch):
            if pos == STREAM_CAP:
                stream += 1
                pos = 0
            chunk_map[(s, c)] = (stream, pos)
            if (s, stream) not in vmap:
                vmap[(s, stream)] = len(variants)
                variants.append((s, stream))
            pos += 1
    assert stream == 7 and pos == STREAM_CAP, (stream, pos)
    return chunk_map, variants, vmap

CHUNK_MAP, VARIANTS, VMAP = _pack_streams()
NVAR = len(VARIANTS)

# ALT segment offsets in QB (per slot), in columns
ALT_OFFS = []
_o = 0
for _c in ALT_CHUNKS:
    ALT_OFFS.append(_o)
    _o += _c * 512
assert _o == QB_COLS

# ---------------------------------------------------------------------------
# Schedule: PE-route reduce groups + engine assignment
# ---------------------------------------------------------------------------

def make_schedule():
    """PE-route groups: (slot, chunk0, nchunks, is_diag, engine)."""
    groups = []
    for s, nch in enumerate(PE_CHUNKS):
        c = 0
        while c < nch:
            if c == 0 and s < 6:
                g = 1            # narrow diag group: shifts reduce work to DVE
            else:
                g = min(2, nch - c)
            groups.append([s, c, g, c == 0])
            c += g
    # diag groups forced to ACT; others balance DVE-heavy (ACT also runs the
    # ACT-direct route, so give DVE everything it can take)
    act_cost = sum(ALT_CHUNKS) * 512 * 0.8333 + 8 * 370.0   # ACT-direct load
    dve_cost = 0.0
    sched = []
    for s, c0, g, diag in groups:
        w = g * 512
        ca = w * 0.8333 + 290.0
        cd = w * 1.0417 + 170.0
        if diag:
            eng = "A"
        else:
            eng = "A" if act_cost + ca <= dve_cost + cd else "D"
        if eng == "A":
            act_cost += ca
        else:
            dve_cost += cd
        sched.append((s, c0, g, diag, eng))
    return sched

SCHEDULE = make_schedule()

# ---------------------------------------------------------------------------
# Device program
# ---------------------------------------------------------------------------

_CACHE = {}

def build_program():
    import concourse.bacc as bacc
    import concourse.mybir as mybir
    from concourse.tile import TileContext

    F32 = mybir.dt.float32
    BF16 = mybir.dt.bfloat16
    AX = mybir.AxisListType
    OP = mybir.AluOpType
    AF = mybir.ActivationFunctionType

    nA = sum(1 for g in SCHEDULE if g[4] == "A") + 8   # + 8 ACT-direct groups
    nD = sum(1 for g in SCHEDULE if g[4] == "D")

    nc = bacc.Bacc(trn_type="TRN2")
    rhs_d = nc.dram_tensor("rhs", [16, STREAM_CAP * 512], BF16, kind="ExternalInput")
    lhs_d = nc.dram_tensor("lhs", [16, NVAR * 128], BF16, kind="ExternalInput")
    tri_d = nc.dram_tensor("tri", [128, 128], BF16, kind="ExternalInput")
    pen_d = nc.dram_tensor("pen", [128, 128], BF16, kind="ExternalInput")
    qb_d = nc.dram_tensor("qb", [128, QB_COLS], BF16, kind="ExternalInput")
    qcol_d = nc.dram_tensor("qcol", [128, 8], F32, kind="ExternalInput")
    lin_d = nc.dram_tensor("linab", [128, 16], F32, kind="ExternalInput")
    out_d = nc.dram_tensor("out", [128, 1], F32, kind="ExternalOutput")

    with TileContext(nc) as tc:
        with tc.tile_pool(name="consts", bufs=1) as cpool, \
             tc.tile_pool(name="scr", bufs=2) as spool, \
             tc.tile_pool(name="ps", bufs=4, space="PSUM") as psp:
            RHS = cpool.tile([16, STREAM_CAP * 512], BF16)
            LHS = cpool.tile([16, NVAR * 128], BF16)
            TRI = cpool.tile([128, 128], BF16)
            PEN = cpool.tile([128, 128], BF16)
            QB = cpool.tile([128, QB_COLS], BF16)
            QCOL = cpool.tile([128, 8], F32)
            LIN = cpool.tile([128, 16], F32)
            ACCA = cpool.tile([128, nA], F32)
            ACCD = cpool.tile([128, max(nD, 1)], F32)
            ACCL = cpool.tile([128, 8], F32)
            R = cpool.tile([128, 4], F32)
            OUT = cpool.tile([128, 1], F32)

            nc.sync.dma_start(out=RHS[:], in_=rhs_d[:])
            nc.sync.dma_start(out=LHS[:], in_=lhs_d[:])
            nc.sync.dma_start(out=TRI[:], in_=tri_d[:])
            nc.sync.dma_start(out=PEN[:], in_=pen_d[:])
            nc.sync.dma_start(out=QCOL[:], in_=qcol_d[:])
            nc.sync.dma_start(out=LIN[:], in_=lin_d[:])
            # QB streamed per-slot so ACT-direct groups start early
            for s in range(8):
                w = ALT_CHUNKS[s] * 512
                if w:
                    nc.sync.dma_start(out=QB[:, ALT_OFFS[s]:ALT_OFFS[s] + w],
                                      in_=qb_d[:, ALT_OFFS[s]:ALT_OFFS[s] + w])

            # dep-free PE warmup while input DMAs are in flight
            DW = cpool.tile([128, 512], BF16)
            nc.gpsimd.memset(DW[:], 0.0)
            WPS = psp.tile([128, 1024], F32, tag="ps")
            for _ in range(4):
                nc.tensor.matmul(WPS[:, 0:512], DW[0:16, 0:128], DW[0:16, 0:512],
                                 start=True, stop=True)

            ia = 0
            id_ = 0
            alt_done = [False] * 8
            for gi, (s, c0, g, diag, eng) in enumerate(SCHEDULE):
                w = g * 512
                PS = psp.tile([128, 1024], F32, tag="ps")
                for b in range(g):
                    st, pos = CHUNK_MAP[(s, c0 + b)]
                    v = VMAP[(s, st)]
                    nc.tensor.matmul(PS[:, b * 512:(b + 1) * 512],
                                     LHS[:, v * 128:(v + 1) * 128],
                                     RHS[:, pos * 512:(pos + 1) * 512],
                                     start=True, stop=not (diag and b == 0))
                if diag:
                    nc.tensor.matmul(PS[:, 0:128], TRI[:], PEN[:],
                                     start=False, stop=True)
                if eng == "A":
                    SCR = spool.tile([128, 1024], F32, tag="scr")
                    nc.scalar.activation(out=SCR[:, :w], in_=PS[:, :w], func=AF.Relu,
                                         bias=0.0, scale=1.0,
                                         accum_out=ACCA[:, ia:ia + 1])
                    ia += 1
                else:
                    nc.vector.tensor_reduce(out=ACCD[:, id_:id_ + 1], in_=PS[:, :w],
                                            axis=AX.X, op=OP.add,
                                            apply_absolute_value=True)
                    id_ += 1
                # interleave ACT-direct groups after this slot's PE groups
                if not alt_done[s]:
                    last_of_slot = all(SCHEDULE[j][0] != s for j in
                                       range(gi + 1, len(SCHEDULE)))
                    if last_of_slot and ALT_CHUNKS[s] > 0:
                        wq = ALT_CHUNKS[s] * 512
                        SCR2 = spool.tile([128, 4096], F32, tag="scr2")
                        nc.scalar.activation(out=SCR2[:, :wq],
                                             in_=QB[:, ALT_OFFS[s]:ALT_OFFS[s] + wq],
                                             func=AF.Relu,
                                             bias=QCOL[:, s:s + 1], scale=1.0,
                                             accum_out=ACCA[:, ia:ia + 1])
                        ia += 1
                        alt_done[s] = True

            # linear terms: accL[:, s] = A_s * q_u + B_s
            for s in range(8):
                nc.vector.tensor_scalar(ACCL[:, s:s + 1], QCOL[:, s:s + 1],
                                        LIN[:, 2 * s:2 * s + 1],
                                        LIN[:, 2 * s + 1:2 * s + 2],
                                        OP.mult, OP.add)

            # combine: out = sum(ACCA) + 0.5*(sum(ACCD) + sum(ACCL))
            nc.vector.tensor_reduce(out=R[:, 0:1], in_=ACCA[:], axis=AX.X, op=OP.add)
            nc.vector.tensor_reduce(out=R[:, 1:2], in_=ACCD[:], axis=AX.X, op=OP.add)
            nc.vector.tensor_reduce(out=R[:, 2:3], in_=ACCL[:], axis=AX.X, op=OP.add)
            nc.vector.tensor_tensor(out=R[:, 1:2], in0=R[:, 1:2], in1=R[:, 2:3],
                                    op=OP.add)
            nc.vector.tensor_scalar(R[:, 1:2], R[:, 1:2], 0.5, None, OP.mult)
            nc.vector.tensor_tensor(out=R[:, 0:1], in0=R[:, 0:1], in1=R[:, 1:2],
                                    op=OP.add)
            nc.vector.tensor_copy(out=OUT[:], in_=R[:, 0:1])
            nc.sync.dma_start(out=out_d[:], in_=OUT[:])

    nc.finalize()
    return nc


def get_program():
    if "nc" not in _CACHE:
        _CACHE["nc"] = build_program()
    return _CACHE["nc"]

# ---------------------------------------------------------------------------
# Host side
# ---------------------------------------------------------------------------

def core_tiles(k):
    return sorted([k + 16 * m for m in range(4)] + [15 - k + 16 * m for m in range(4)])


def build_inputs(q):
    """Per-core in_maps for label-sorted preds q (np.float32 [8192])."""
    import ml_dtypes
    BF = ml_dtypes.bfloat16
    q = q.astype(np.float32)
    qb16 = q.astype(BF)
    rhs1_full = (2.0 - q).astype(np.float32).astype(BF)
    tri = np.triu(np.ones((128, 128), np.float32)).astype(BF)
    pen = np.zeros((128, 128), np.float32)
    pen[np.arange(128), np.arange(128)] = PENALTY
    pen = pen.astype(BF)

    in_maps = []
    for k in range(8):
        tiles = core_tiles(k)
        rhs = np.zeros((16, STREAM_CAP * 512), BF)
        lhs = np.zeros((16, NVAR * 128), BF)
        qbt = np.full((128, QB_COLS), PAD_VAL, np.float32).astype(BF)
        qcol = np.zeros((128, 8), np.float32)
        lin = np.zeros((128, 16), np.float32)
        for s, t in enumerate(tiles):
            real = (NBLK - t) * 128
            qcol[:, s] = qb16[t * 128:(t + 1) * 128].astype(np.float32)
            # PE-route chunks
            for c in range(PE_CHUNKS[s]):
                st, pos = CHUNK_MAP[(s, c)]
                lo = c * 512
                take = min(max(real - lo, 0), 512)
                if take > 0:
                    rhs[2 * st, pos * 512: pos * 512 + take] = np.float32(1.0)
                    rhs[2 * st + 1, pos * 512: pos * 512 + take] = \
                        rhs1_full[t * 128 + lo: t * 128 + lo + take]
                v = VMAP[(s, st)]
                lhs[2 * st, v * 128:(v + 1) * 128] = qb16[t * 128:(t + 1) * 128]
                lhs[2 * st + 1, v * 128:(v + 1) * 128] = np.float32(1.0)
            # ACT-direct chunks (tail of the window)
            for a in range(ALT_CHUNKS[s]):
                lo = (PE_CHUNKS[s] + a) * 512
                take = min(max(real - lo, 0), 512)
                col0 = ALT_OFFS[s] + a * 512
                if take > 0:
                    qbt[:, col0:col0 + take] = \
                        rhs1_full[t * 128 + lo: t * 128 + lo + take][None, :]
            # linear terms over this slot's DVE groups
            A = 0.0
            B = 0.0
            for (gs, c0, g, diag, eng) in SCHEDULE:
                if gs != s or eng != "D":
                    continue
                for b in range(g):
                    st, pos = CHUNK_MAP[(s, c0 + b)]
                    A += rhs[2 * st, pos * 512:(pos + 1) * 512].astype(np.float64).sum()
                    B += rhs[2 * st + 1, pos * 512:(pos + 1) * 512].astype(np.float64).sum()
            lin[:, 2 * s] = np.float32(A)
            lin[:, 2 * s + 1] = np.float32(B)
        in_maps.append({"rhs": rhs, "lhs": lhs, "tri": tri, "pen": pen,
                        "qb": qbt, "qcol": qcol, "linab": lin})
    return in_maps


def emulate(in_maps):
    """Numpy emulation of the device program (for offline validation)."""
    total = 0.0
    for k in range(8):
        m = in_maps[k]
        rhs = m["rhs"].astype(np.float32)
        lhs = m["lhs"].astype(np.float32)
        tri = m["tri"].astype(np.float32)
        pen = m["pen"].astype(np.float32)
        qb = m["qb"].astype(np.float32)
        qcol = m["qcol"]
        lin = m["linab"]
        accA = 0.0
        accD = 0.0
        accL = 0.0
        for (s, c0, g, diag, eng) in SCHEDULE:
            ps = np.zeros((128, g * 512), np.float64)
            for b in range(g):
                st, pos = CHUNK_MAP[(s, c0 + b)]
                v = VMAP[(s, st)]
                L = lhs[:, v * 128:(v + 1) * 128]
                Rr = rhs[:, pos * 512:(pos + 1) * 512]
                ps[:, b * 512:(b + 1) * 512] = L.T @ Rr
            if diag:
                ps[:, 0:128] += tri.T @ pen
            if eng == "A":
                accA += np.maximum(ps, 0).sum()
            else:
                accD += np.abs(ps).sum()
        for s in range(8):
            wq = ALT_CHUNKS[s] * 512
            if wq:
                t = qb[:, ALT_OFFS[s]:ALT_OFFS[s] + wq] + qcol[:, s][:, None]
                accA += np.maximum(t, 0).sum()
            accL += (lin[0, 2 * s] * qcol[:, s] + lin[0, 2 * s + 1]).sum()
        total += accA + 0.5 * (accD + accL)
    return total


def tie_correction(labels, q, order):
    ls = labels[order]
    corr = 0.0
    i = 0
    n = len(ls)
    while i < n:
        j = i + 1
        while j < n and ls[j] == ls[i]:
            j += 1
        if j - i > 1:
            for u in range(i, j):
                for v in range(u + 1, j):
                    corr += 2.0 - max(0.0, 2.0 + float(q[u]) - float(q[v]))
        i = j
    return corr


def run(inputs, trace=False):
    from concourse.bass_utils import run_bass_kernel_spmd

    preds = np.asarray(inputs["preds"], dtype=np.float32)
    labels = np.asarray(inputs["labels"], dtype=np.float32)
    order = np.argsort(labels, kind="stable")
    q = preds[order]

    nc = get_program()
    in_maps = build_inputs(q)
    res = run_bass_kernel_spmd(nc, in_maps, core_ids=list(range(8)), trace=trace)
    total = 0.0
    for c in range(8):
        total += res.results[c]["out"].astype(np.float64).sum()
    total += tie_correction(labels, q, order)
    return np.float32(total), res


def kernel(**inputs):
    out, _ = run(inputs, trace=False)
    return out
